# revision 14
# baseline (speedup 1.0000x reference)
"""Self-contained Trainium2 (Bass) kernel for the BaseSigKernel problem.

kernel(xs, ys) -> (24, 24) float32 signature-kernel Gram matrix.

Math (per (x,y) pair; Salvi et al. finite-difference scheme, dyadic_order=1):
    a[r, s]   = <dy[r], dx[s]> / 4          (190x190, dyadic 2x2-duplicated)
    c1 = 1 + a/2 + a^2/12 ;  c2 = 1 - a^2/12
    u[0, :] = u[:, 0] = 1
    u[r+1, s+1] = (u[r+1, s] + u[r, s+1]) * c1[r, s] - u[r, s] * c2[r, s]
    result = u[190, 190]

Distribution: data-parallel over batch_x - core ci owns b in {3ci..3ci+2} x
all 24 c's = 72 pairs on SBUF partitions (three 32-bands, 24 used each).

Per core the 190 PDE rows are serial; each row is one interleaved DVE
tensor_tensor_scan of 380 steps (2/cell: state += t[s+1]; state =
c1*state + (-c2*t[s])). The -c2[s]*t[s] products go into the dead even
lanes of the previous row's buffer: DVE computes cells 0..M-1, the Pool
engine cells M..189 concurrently (Pool cannot scan on TRN2 - ISA check -
but tensor_tensor works). Coefficients are produced ahead in 5-coarse-row
chunks: PE matmuls (K=8) -> PSUM, ScalarE square/affine -> s12/t2,
c1 = t2+s12 sliced into small DVE adds placed as spacers between the
row ops (they hide same-engine RAW semaphore latency), ScalarE expands
with dyadic column duplication -> full-res scan operands.

u rows (8 rotating width-388 tiles): u[k] at 2k+1, products at 2s+4.
"""

import math
from contextlib import ExitStack

import numpy as np

import concourse.bacc as bacc
import concourse.mybir as mybir
import concourse.tile as tile
from concourse.ap import AP

F32 = mybir.dt.float32
Alu = mybir.AluOpType
Act = mybir.ActivationFunctionType

BX, BY, L, DIM = 24, 24, 96, 8
N_CORES = 8
BB = BX // N_CORES          # 3 b-values per core
BAND = 32                   # matmul output base partitions must be 0/32/64
P = BB * BAND               # 96 partitions
NH = L - 1                  # 95 half-resolution grid length
NF = 2 * NH                 # 190 full-resolution grid length
INV_SQRT12 = 1.0 / math.sqrt(12.0)
UW = 388                    # u row buffer width
MSPLIT = 148                # product cells 0..MSPLIT-1 on DVE, rest on Pool
CH = 5                      # coarse rows per coefficient chunk
NCH = NH // CH              # 19 chunks
RING = 4                    # coefficient ring slots
NPIECE = 20                 # c1h add slices per chunk (2 per row)


def _view(t_ap: AP, off: int, dims) -> AP:
    return AP(t_ap.tensor, t_ap.offset + off, [list(d) for d in dims])


def build_bass(msplit: int = MSPLIT):
    m = msplit
    assert 2 <= m <= NF
    nc = bacc.Bacc()
    inp_d = nc.declare_dram_parameter("inp", [DIM, NH * BAND + BB * NH], F32, isOutput=False)
    out_d = nc.declare_dram_parameter("out", [P, 1], F32, isOutput=True)

    with ExitStack() as ctx:
        tc = ctx.enter_context(tile.TileContext(nc))
        sbuf = ctx.enter_context(tc.tile_pool(name="sbuf", bufs=1))
        psum = ctx.enter_context(tc.tile_pool(name="psum", bufs=RING, space="PSUM"))

        inp_t = sbuf.tile([DIM, NH * BAND + BB * NH], F32, name="inp_t", tag="inp_t")
        nc.gpsimd.dma_start(inp_t[:], inp_d[:])

        ub = [sbuf.tile([P, UW], F32, name=f"u{i}", tag=f"u{i}") for i in range(8)]
        for t in ub:
            nc.vector.memset(t[:], 1.0)

        d0s = [sbuf.tile([P, CH * 2 * NF], F32, name=f"d0_{i}", tag=f"d0_{i}") for i in range(RING)]
        c2s = [sbuf.tile([P, CH * NF], F32, name=f"c2_{i}", tag=f"c2_{i}") for i in range(RING)]
        s12s = [sbuf.tile([P, CH * NH], F32, name=f"s12_{i}", tag=f"s12_{i}") for i in range(RING)]
        t2s = [sbuf.tile([P, CH * NH], F32, name=f"t2_{i}", tag=f"t2_{i}") for i in range(RING)]
        c1hs = [sbuf.tile([P, CH * NH], F32, name=f"c1h_{i}", tag=f"c1h_{i}") for i in range(RING)]

        # scan data0 even lanes are the constant 1.0 forever
        for d0 in d0s:
            ps, _ = d0.ap[0]
            nc.gpsimd.memset(_view(d0, 0, [(ps, P), (2, CH * NF)]), 1.0)

        cbias = sbuf.tile([P, 1], F32, name="cbias", tag="cbias")
        nc.gpsimd.memset(cbias[:], -1.0)

        W = CH * NH

        def fine_ops(c, dq):
            """Production for coarse row q=c*CH+dq: 3 matmuls, s12, t2,
            c1h add (DVE), c2neg+c1 expands. Fine granularity lets the
            scheduler pipeline production against the scan rows."""
            slot = c % RING
            q = c * CH + dq
            pa = pas[c]
            s12, t2, c1h = s12s[slot], t2s[slot], c1hs[slot]
            d0, c2 = d0s[slot], c2s[slot]
            lo, hi = dq * NH, (dq + 1) * NH
            ps_c2, _ = c2.ap[0]
            ps_s, _ = s12.ap[0]
            ps_d0, _ = d0.ap[0]
            ps_c1, _ = c1h.ap[0]
            lhsT = inp_t[:, q * BAND : (q + 1) * BAND]
            ops = []
            for b in range(BB):
                ops.append(lambda b=b, pa=pa, lhsT=lhsT: nc.tensor.matmul(
                    pa[b * BAND : (b + 1) * BAND, lo:hi],
                    lhsT,
                    inp_t[:, NH * BAND + b * NH : NH * BAND + (b + 1) * NH],
                ))
            ops.append(lambda: nc.scalar.activation(
                s12[:, lo:hi], pa[:, lo:hi], Act.Square, scale=INV_SQRT12))
            ops.append(lambda: nc.scalar.activation(
                t2[:, lo:hi], pa[:, lo:hi], Act.Identity, bias=1.0, scale=0.5))
            ops.append(lambda: nc.vector.tensor_tensor(
                c1h[:, lo:hi], t2[:, lo:hi], s12[:, lo:hi], Alu.add))
            ops.append(lambda: nc.scalar.activation(
                _view(c2, 2 * lo, [(ps_c2, P), (2, NH), (1, 2)]),
                _view(s12, lo, [(ps_s, P), (1, NH), (0, 2)]),
                Act.Identity, bias=cbias[:]))
            ops.append(lambda: nc.scalar.activation(
                _view(d0, 4 * lo + 1, [(ps_d0, P), (4, NH), (2, 2)]),
                _view(c1h, lo, [(ps_c1, P), (1, NH), (0, 2)]),
                Act.Copy))
            return ops

        # one psum tile per chunk (allocated in order; pool rotates RING slots)
        pas = {}
        for c in range(NCH):
            pas[c] = psum.tile([P, 512], F32, name="pa", tag="pa")

        # prologue: production for the first 2 chunks + 1 coarse row
        emitted = 0
        fine_all = [(c, dq) for c in range(NCH) for dq in range(CH)]
        for c, dq in fine_all[: 2 * CH + 1]:
            for op in fine_ops(c, dq):
                op()
            emitted += 1

        for r in range(NF):
            q = r // 2
            c = q // CH
            dq = q % CH
            slot = c % RING

            # stay 2*CH+1 coarse rows ahead with production
            want = min(q + 2 * CH + 2, NH)
            while emitted < want:
                cc, dqq = fine_all[emitted]
                for op in fine_ops(cc, dqq):
                    op()
                emitted += 1

            up = ub[r % 8]
            un = ub[(r + 1) % 8]
            us, _ = up.ap[0]
            d0row = d0s[slot][:, dq * 2 * NF : (dq + 1) * 2 * NF]
            c2row = c2s[slot][:, dq * NF : (dq + 1) * NF]

            # Pool: products -c2[s]*t[s] for s=m..189 at even slots 2s+4
            if m < NF:
                nc.gpsimd.tensor_tensor(
                    _view(up, 2 * m + 4, [(us, P), (2, NF - m)]),
                    c2row[:, m:NF],
                    _view(up, 2 * m + 1, [(us, P), (2, NF - m)]),
                    Alu.mult,
                )
            # DVE: products for s=0..m-1
            nc.vector.tensor_tensor(
                _view(up, 4, [(us, P), (2, m)]),
                c2row[:, 0:m],
                _view(up, 1, [(us, P), (2, m)]),
                Alu.mult,
            )
            # the full interleaved scan: 380 steps, u[k] -> un[2k+1]
            nc.vector.tensor_tensor_scan(
                un[:, 2 : 2 + 2 * NF],
                d0row[:, 0 : 2 * NF],
                up[:, 3 : 3 + 2 * NF],
                1.0,
                Alu.mult,
                Alu.add,
            )

        nc.gpsimd.dma_start(out_d[:], ub[NF % 8][:, 2 * NF + 1 : 2 * NF + 2])

    nc.compile()
    return nc


def pack_inputs(xs: np.ndarray, ys: np.ndarray):
    """Full inputs -> per-core in_maps for run_bass_kernel_spmd."""
    xs = np.asarray(xs, np.float32)
    ys = np.asarray(ys, np.float32)
    dx = np.diff(xs, axis=1) * 0.5            # (24, 95, 8)
    dy = np.diff(ys, axis=1) * 0.5            # (24, 95, 8)
    dyT = np.zeros((DIM, NH, BAND), np.float32)
    dyT[:, :, :BY] = dy.transpose(2, 1, 0)
    dyT = dyT.reshape(DIM, NH * BAND)
    in_maps = []
    for ci in range(N_CORES):
        dxc = dx[ci * BB : (ci + 1) * BB]     # (3, 95, 8)
        dxT = dxc.transpose(2, 0, 1).reshape(DIM, BB * NH)
        inp = np.ascontiguousarray(np.concatenate([dyT, dxT], axis=1))
        in_maps.append({"inp": inp})
    return in_maps


def unpack_outputs(results) -> np.ndarray:
    out = np.zeros((BX, BY), np.float32)
    for ci in range(N_CORES):
        res = np.asarray(results[ci]["out"]).reshape(P)
        for b in range(BB):
            out[ci * BB + b, :] = res[b * BAND : b * BAND + BY]
    return out


_NC_CACHE = None


def kernel(xs: np.ndarray, ys: np.ndarray) -> np.ndarray:
    """Full (24,96,8) inputs -> full (24,24) output, computed on 8 trn2 cores."""
    global _NC_CACHE
    from concourse.bass_utils import run_bass_kernel_spmd

    if _NC_CACHE is None:
        _NC_CACHE = build_bass()
    in_maps = pack_inputs(xs, ys)
    r = run_bass_kernel_spmd(_NC_CACHE, in_maps, list(range(N_CORES)))
    return unpack_outputs(r.results)
